# revision 1
# baseline (speedup 1.0000x reference)
"""RNN-T Joiner kernel for Trainium2 (Bass/Tile), 8-core data-parallel over batch.

out[b,t,u,v] = (enc[b,t] @ We)[v] + (pred[b,u] @ Wp)[v] + bias[v]

Per core (one batch element):
  - PE (fp32): enc_proj [256,1024] and pred_b [65,1024] projections.
  - PE (fp32r): broadcast pred_b rows across the 128 t-partitions via one-hot
    selection matmuls into PSUM. Even u rows live at partitions 0-32, odd u
    rows at partitions 64-95, so consecutive matmuls alternate PE row groups
    and LDWEIGHTS overlaps in-flight MATMULs (64-deep reorder window).
  - DVE: one tensor_tensor add per output element (the mandatory PSUM->SBUF
    trip) producing staged output tiles.
  - HWDGE DMA: 10 uniform 6.8 MB contiguous stores (13 u's per block).
"""

import sys

sys.path.insert(0, "/opt/trn_rl_repo")

import numpy as np

B, T, U1, D, V = 8, 256, 65, 640, 1024
KC = D // 128  # 5 contraction chunks
UBLK = 13      # u's per output DMA block: 5 blocks x 13 = 65
NBLK = U1 // UBLK
NE = (U1 + 1) // 2  # 33 even u rows (0,2,..,64)
NO = U1 // 2        # 32 odd u rows (1,3,..,63)

_COMPILED = None


def _build():
    import concourse.bacc as bacc
    import concourse.tile as tile
    import concourse.mybir as mybir

    f32 = mybir.dt.float32
    f32r = mybir.dt.float32r

    nc = bacc.Bacc("TRN2", target_bir_lowering=False, debug=False, num_devices=8)

    encT = nc.dram_tensor("encT", [D, T], f32, kind="ExternalInput")
    # predT columns: even u's (0,2,..,64) then odd u's (1,3,..,63)
    predT = nc.dram_tensor("predT", [D, U1], f32, kind="ExternalInput")
    W = nc.dram_tensor("W", [2 * D, V], f32, kind="ExternalInput")
    bias = nc.dram_tensor("bias", [1, V], f32, kind="ExternalInput")
    ones = nc.dram_tensor("ones", [1, 128], f32, kind="ExternalInput")
    # packed one-hot: rows 0-32 select even u (identity33 x ones128),
    # rows 64-95 select odd u (identity32 x ones128)
    sel = nc.dram_tensor("sel", [128, NE * 128], f32r, kind="ExternalInput")
    out = nc.dram_tensor("out", [T, U1 * V], f32, kind="ExternalOutput")

    with tile.TileContext(nc) as tc:
        with tc.tile_pool(name="consts", bufs=1) as cp:
            sel_sb = cp.tile([128, NE * 128], f32r, tag="sel")
            pred_sp = cp.tile([128, V], f32r, tag="pred_sp")
            enc_dup = []
            for tt in range(2):
                t_ = cp.tile([128, 2 * V], f32, tag=f"enc_dup{tt}")
                enc_dup.append(t_)

            with tc.tile_pool(name="wpool", bufs=1) as wp:
                predT_sb = []
                Wp_sb = []
                encT_sb = []
                We_sb = []
                for c in range(KC):
                    t_ = wp.tile([128, U1], f32, tag=f"predT{c}")
                    nc.sync.dma_start(t_[:], predT[c * 128:(c + 1) * 128, :])
                    predT_sb.append(t_)
                    t_ = wp.tile([128, V], f32, tag=f"Wp{c}")
                    nc.sync.dma_start(t_[:], W[D + c * 128:D + (c + 1) * 128, :])
                    Wp_sb.append(t_)
                bias_sb = wp.tile([1, V], f32, tag="bias")
                nc.sync.dma_start(bias_sb[:], bias[:])
                ones_sb = wp.tile([1, 128], f32, tag="ones")
                nc.sync.dma_start(ones_sb[:], ones[:])
                for c in range(KC):
                    t_ = wp.tile([128, T], f32, tag=f"encT{c}")
                    nc.sync.dma_start(t_[:], encT[c * 128:(c + 1) * 128, :])
                    encT_sb.append(t_)
                    t_ = wp.tile([128, V], f32, tag=f"We{c}")
                    nc.sync.dma_start(t_[:], W[c * 128:(c + 1) * 128, :])
                    We_sb.append(t_)
                nc.sync.dma_start(sel_sb[:], sel[:])

                # ---- setup: projections (fp32 PE matmuls) ----
                with tc.tile_pool(name="spsum", bufs=2, space="PSUM") as sp:
                    ps_p = sp.tile([128, V], f32, tag="ps")
                    for vt in range(2):
                        vs = slice(vt * 512, (vt + 1) * 512)
                        for c in range(KC):
                            nc.tensor.matmul(
                                ps_p[0:NE, vs], predT_sb[c][:, 0:NE],
                                Wp_sb[c][:, vs], start=(c == 0), stop=False)
                        nc.tensor.matmul(
                            ps_p[0:NE, vs], ones_sb[0:1, 0:NE], bias_sb[0:1, vs],
                            start=False, stop=True)
                    for vt in range(2):
                        vs = slice(vt * 512, (vt + 1) * 512)
                        for c in range(KC):
                            nc.tensor.matmul(
                                ps_p[64:64 + NO, vs], predT_sb[c][:, NE:U1],
                                Wp_sb[c][:, vs], start=(c == 0), stop=False)
                        nc.tensor.matmul(
                            ps_p[64:64 + NO, vs], ones_sb[0:1, 0:NO], bias_sb[0:1, vs],
                            start=False, stop=True)
                    nc.vector.tensor_copy(pred_sp[0:NE, :], ps_p[0:NE, :])
                    nc.vector.tensor_copy(pred_sp[64:64 + NO, :], ps_p[64:64 + NO, :])

                    for tt in range(2):
                        ts_ = slice(tt * 128, (tt + 1) * 128)
                        ps_e = sp.tile([128, V], f32, tag="pse")
                        for vt in range(2):
                            vs = slice(vt * 512, (vt + 1) * 512)
                            for c in range(KC):
                                nc.tensor.matmul(
                                    ps_e[:, vs], encT_sb[c][:, ts_], We_sb[c][:, vs],
                                    start=(c == 0), stop=(c == KC - 1))
                        nc.vector.tensor_copy(enc_dup[tt][:, 0:V], ps_e[:])
                        nc.vector.tensor_copy(enc_dup[tt][:, V:2 * V], ps_e[:])

            def bcast_mm(ps_ap, u, vt):
                # one [128,512] slice of pred_b[u] broadcast to all partitions
                vs = slice(vt * 512, (vt + 1) * 512)
                if u % 2 == 0:
                    nc.tensor.matmul(
                        ps_ap, sel_sb[0:NE, (u // 2) * 128:(u // 2 + 1) * 128],
                        pred_sp[0:NE, vs], start=True, stop=True)
                else:
                    nc.tensor.matmul(
                        ps_ap, sel_sb[64:64 + NO, (u // 2) * 128:(u // 2 + 1) * 128],
                        pred_sp[64:64 + NO, vs], start=True, stop=True)

            # ---- main loop: broadcast-add-store ----
            # psum broadcast tiles are identical for both t-halves: compute
            # once, add into both t-stages (halves PE work).
            with tc.tile_pool(name="outp", bufs=2) as op_, \
                 tc.tile_pool(name="mpsum", bufs=2, space="PSUM") as mp:
                for blk in range(9):
                    u0 = blk * 8
                    nu = 8 if blk < 7 else 4
                    if blk == 8:
                        u0 = 60
                    stage0 = op_.tile([128, 8 * V], f32, tag="stage0")
                    stage1 = op_.tile([128, 8 * V], f32, tag="stage1")
                    for pair in range(nu // 2):
                        ua = u0 + 2 * pair
                        ps = mp.tile([128, 2048], f32, tag="mps")
                        bcast_mm(ps[:, 0:512], ua, 0)
                        bcast_mm(ps[:, 1024:1536], ua + 1, 0)
                        bcast_mm(ps[:, 512:1024], ua, 1)
                        bcast_mm(ps[:, 1536:2048], ua + 1, 1)
                        nc.vector.tensor_add(
                            stage0[:, pair * 2048:(pair + 1) * 2048],
                            enc_dup[0][:], ps[:])
                        nc.vector.tensor_add(
                            stage1[:, pair * 2048:(pair + 1) * 2048],
                            enc_dup[1][:], ps[:])
                    nc.sync.dma_start(
                        out[0:128, u0 * V:(u0 + nu) * V], stage0[:, 0:nu * V])
                    nc.sync.dma_start(
                        out[128:256, u0 * V:(u0 + nu) * V], stage1[:, 0:nu * V])
                # tail u = 64
                u = U1 - 1
                stage0 = op_.tile([128, 8 * V], f32, tag="stage0")
                stage1 = op_.tile([128, 8 * V], f32, tag="stage1")
                ps = mp.tile([128, 2048], f32, tag="mps")
                bcast_mm(ps[:, 0:512], u, 0)
                bcast_mm(ps[:, 512:1024], u, 1)
                nc.vector.tensor_add(stage0[:, 0:V], enc_dup[0][:, 0:V], ps[:, 0:V])
                nc.vector.tensor_add(stage1[:, 0:V], enc_dup[1][:, 0:V], ps[:, 0:V])
                nc.sync.dma_start(out[0:128, u * V:(u + 1) * V], stage0[:, 0:V])
                nc.sync.dma_start(out[128:256, u * V:(u + 1) * V], stage1[:, 0:V])

    nc.compile()
    return nc


def _get_compiled():
    global _COMPILED
    if _COMPILED is None:
        _COMPILED = _build()
    return _COMPILED


def _in_maps(encoder_out, predictor_out, W, b):
    sel = np.zeros((128, NE * 128), dtype=np.float32)
    for r in range(NE):
        sel[r, r * 128:(r + 1) * 128] = 1.0      # selects even u = 2r
    for r in range(NO):
        sel[64 + r, r * 128:(r + 1) * 128] = 1.0  # selects odd u = 2r+1
    ones = np.ones((1, 128), dtype=np.float32)
    bias = np.ascontiguousarray(b.reshape(1, V).astype(np.float32))
    Wc = np.ascontiguousarray(W.astype(np.float32))
    eo = list(range(0, U1, 2)) + list(range(1, U1, 2))
    maps = []
    for i in range(B):
        pT = predictor_out[i].T.astype(np.float32)  # [D, U1]
        maps.append({
            "encT": np.ascontiguousarray(encoder_out[i].T.astype(np.float32)),
            "predT": np.ascontiguousarray(pT[:, eo]),
            "W": Wc,
            "bias": bias,
            "ones": ones,
            "sel": sel,
        })
    return maps


def run(encoder_out, predictor_out, W, b, trace=False, tmpdir=None):
    from concourse.bass_utils import run_bass_kernel_spmd

    nc = _get_compiled()
    maps = _in_maps(encoder_out, predictor_out, W, b)
    res = run_bass_kernel_spmd(
        nc, maps, list(range(B)), trace=trace,
        **({"tmpdir": tmpdir} if tmpdir else {}))
    outs = np.stack([res.results[i]["out"].reshape(T, U1, V) for i in range(B)])
    return outs, res


def kernel(encoder_out, predictor_out, W, b):
    outs, _ = run(encoder_out, predictor_out, W, b)
    return outs



# revision 2
# speedup vs baseline: 1.8525x; 1.8525x over previous
"""RNN-T Joiner kernel for Trainium2 (Bass/Tile), 8-core data-parallel over batch.

out[b,t,u,v] = (enc[b,t] @ We)[v] + (pred[b,u] @ Wp)[v] + bias[v]

Layout trick: put V on SBUF partitions. Then for a fixed u, the pred term is a
per-partition scalar, so the broadcast-add is a DVE tensor_scalar_add in 4x
perf mode (fp16, all-SBUF) instead of a PE one-hot broadcast + fp32
tensor_tensor add. The fp16 datapath also halves HBM store traffic (34 MB/core
vs 68 MB), which is the roofline term. Tolerance is 2e-2 rel; fp16 rounding
contributes ~1e-3.

Per core (one batch element):
  - PE (fp16): enc_projT [v,t] and pred_projT [v,u] with V on output
    partitions; bias folded into pred_projT via a ones-row matmul.
  - Act: PSUM->SBUF evacuation (enc_projT cast to fp16, pred_projT kept fp32
    for the exempt per-partition scalar operand).
  - DVE: 520 tensor_scalar_add ops [128, 256] in 4x mode.
  - HWDGE DMA: 5 stores of 6.8 MB, HBM layout [v_lo, v_chunk, u, t],
    un-permuted on the host.
"""

import sys

sys.path.insert(0, "/opt/trn_rl_repo")

import numpy as np

B, T, U1, D, V = 8, 256, 65, 640, 1024
KC = D // 128   # 5 contraction chunks
VC = V // 128   # 8 vocab chunks
NU = 13         # u's per store block: 5 blocks x 13 = 65
NBLK = U1 // NU

_COMPILED = None


def _build():
    import concourse.bacc as bacc
    import concourse.tile as tile
    import concourse.mybir as mybir

    f16 = mybir.dt.float16
    f32 = mybir.dt.float32

    nc = bacc.Bacc("TRN2", target_bir_lowering=False, debug=False, num_devices=8)

    encT = nc.dram_tensor("encT", [D, T], f16, kind="ExternalInput")
    predT = nc.dram_tensor("predT", [D, U1], f16, kind="ExternalInput")
    W = nc.dram_tensor("W", [2 * D, V], f16, kind="ExternalInput")
    bias = nc.dram_tensor("bias", [1, V], f16, kind="ExternalInput")
    ones = nc.dram_tensor("ones", [1, U1], f16, kind="ExternalInput")
    # out[v_lo, v_chunk, u, t] ; v = v_chunk*128 + v_lo
    out = nc.dram_tensor("out", [128, VC, U1, T], f16, kind="ExternalOutput")

    with tile.TileContext(nc) as tc:
        with tc.tile_pool(name="consts", bufs=1) as cp:
            encT_sb, predT_sb, We_sb, Wp_sb = [], [], [], []
            for c in range(KC):
                t_ = cp.tile([128, T], f16, tag=f"encT{c}")
                nc.sync.dma_start(t_[:], encT[c * 128:(c + 1) * 128, :])
                encT_sb.append(t_)
            for c in range(KC):
                t_ = cp.tile([128, V], f16, tag=f"We{c}")
                nc.sync.dma_start(t_[:], W[c * 128:(c + 1) * 128, :])
                We_sb.append(t_)
            for c in range(KC):
                t_ = cp.tile([128, U1], f16, tag=f"predT{c}")
                nc.sync.dma_start(t_[:], predT[c * 128:(c + 1) * 128, :])
                predT_sb.append(t_)
            for c in range(KC):
                t_ = cp.tile([128, V], f16, tag=f"Wp{c}")
                nc.sync.dma_start(t_[:], W[D + c * 128:D + (c + 1) * 128, :])
                Wp_sb.append(t_)
            bias_sb = cp.tile([1, V], f16, tag="bias")
            nc.sync.dma_start(bias_sb[:], bias[:])
            ones_sb = cp.tile([1, U1], f16, tag="ones")
            nc.sync.dma_start(ones_sb[:], ones[:])

            encP = cp.tile([128, VC * T], f16, tag="encP")      # enc_projT[v, t]
            predP = cp.tile([128, VC * U1], f32, tag="predP")   # pred_projT[v, u] + b[v]

            # ---- projections: V on output partitions ----
            with tc.tile_pool(name="ppool", bufs=2, space="PSUM") as pp:
                for vc in range(VC):
                    vs = slice(vc * 128, (vc + 1) * 128)
                    pse = pp.tile([128, T], f32, tag="pse")
                    for c in range(KC):
                        nc.tensor.matmul(
                            pse[:], We_sb[c][:, vs], encT_sb[c][:],
                            start=(c == 0), stop=(c == KC - 1))
                    nc.scalar.copy(encP[:, vc * T:(vc + 1) * T], pse[:])
                    psp = pp.tile([128, U1], f32, tag="psp")
                    for c in range(KC):
                        nc.tensor.matmul(
                            psp[:], Wp_sb[c][:, vs], predT_sb[c][:],
                            start=(c == 0), stop=False)
                    nc.tensor.matmul(
                        psp[:], bias_sb[0:1, vs], ones_sb[0:1, :],
                        start=False, stop=True)
                    nc.scalar.copy(predP[:, vc * U1:(vc + 1) * U1], psp[:])

            # ---- main loop: per-u scalar-add, big interleaved stores ----
            with tc.tile_pool(name="outp", bufs=2) as op_:
                for blk in range(NBLK):
                    u0 = blk * NU
                    stage = op_.tile([128, VC, NU, T], f16, tag="stage")
                    for vc in range(VC):
                        for ui in range(NU):
                            u = u0 + ui
                            nc.vector.tensor_scalar_add(
                                stage[:, vc, ui, :],
                                encP[:, vc * T:(vc + 1) * T],
                                predP[:, vc * U1 + u:vc * U1 + u + 1])
                    nc.sync.dma_start(out[:, :, u0:u0 + NU, :], stage[:])

    nc.compile()
    return nc


def _get_compiled():
    global _COMPILED
    if _COMPILED is None:
        _COMPILED = _build()
    return _COMPILED


def _in_maps(encoder_out, predictor_out, W, b):
    Wc = np.ascontiguousarray(np.asarray(W, dtype=np.float16))
    bias = np.ascontiguousarray(np.asarray(b, dtype=np.float16).reshape(1, V))
    ones = np.ones((1, U1), dtype=np.float16)
    maps = []
    for i in range(B):
        maps.append({
            "encT": np.ascontiguousarray(
                np.asarray(encoder_out[i], dtype=np.float16).T),
            "predT": np.ascontiguousarray(
                np.asarray(predictor_out[i], dtype=np.float16).T),
            "W": Wc,
            "bias": bias,
            "ones": ones,
        })
    return maps


def run(encoder_out, predictor_out, W, b, trace=False, tmpdir=None):
    from concourse.bass_utils import run_bass_kernel_spmd

    nc = _get_compiled()
    maps = _in_maps(encoder_out, predictor_out, W, b)
    res = run_bass_kernel_spmd(
        nc, maps, list(range(B)), trace=trace,
        **({"tmpdir": tmpdir} if tmpdir else {}))
    outs = np.empty((B, T, U1, V), dtype=np.float32)
    for i in range(B):
        arr = res.results[i]["out"]  # [128, VC, U1, T] fp16
        outs[i] = arr.transpose(3, 2, 1, 0).reshape(T, U1, V).astype(np.float32)
    return outs, res


def kernel(encoder_out, predictor_out, W, b):
    outs, _ = run(encoder_out, predictor_out, W, b)
    return outs


# revision 6
# speedup vs baseline: 2.0252x; 1.0932x over previous
"""RNN-T Joiner kernel for Trainium2 (Bass/Tile), 8-core data-parallel over batch.

out[b,t,u,v] = (enc[b,t] @ We)[v] + (pred[b,u] @ Wp)[v] + bias[v]

Layout trick: put V on SBUF partitions. Then for a fixed u, the pred term is a
per-partition scalar, so the broadcast-add is a DVE tensor_scalar_add in 4x
perf mode (fp16, all-SBUF) instead of a PE one-hot broadcast + fp32
tensor_tensor add. The fp16 datapath also halves HBM store traffic (34 MB/core
vs 68 MB), which is the roofline term. Tolerance is 2e-2 rel; fp16 rounding
contributes ~1e-3.

Per core (one batch element):
  - PE (fp16): enc_projT [v,t] and pred_projT [v,u] with V on output
    partitions; bias folded into pred_projT via a ones-row matmul.
  - Act: PSUM->SBUF evacuation (enc_projT cast to fp16, pred_projT kept fp32
    for the exempt per-partition scalar operand).
  - DVE: 520 tensor_scalar_add ops [128, 256] in 4x mode.
  - HWDGE DMA: 5 stores of 6.8 MB, HBM layout [v_lo, v_chunk, u, t],
    un-permuted on the host.
"""

import sys

sys.path.insert(0, "/opt/trn_rl_repo")

import numpy as np

B, T, U1, D, V = 8, 256, 65, 640, 1024
KC = D // 128   # 5 contraction chunks
VC = V // 128   # 8 vocab chunks
NU = 13         # u's per store block: 5 blocks x 13 = 65
NBLK = U1 // NU

_COMPILED = None


def _build():
    import concourse.bacc as bacc
    import concourse.tile as tile
    import concourse.mybir as mybir

    f16 = mybir.dt.float16
    f32 = mybir.dt.float32

    nc = bacc.Bacc("TRN2", target_bir_lowering=False, debug=False, num_devices=8)

    encT = nc.dram_tensor("encT", [D, T], f16, kind="ExternalInput")
    predT = nc.dram_tensor("predT", [D, U1], f16, kind="ExternalInput")
    W = nc.dram_tensor("W", [2 * D, V], f16, kind="ExternalInput")
    bias = nc.dram_tensor("bias", [1, V], f16, kind="ExternalInput")
    ones = nc.dram_tensor("ones", [1, U1], f16, kind="ExternalInput")
    # out[v_lo, v_chunk, u, t] ; v = v_chunk*128 + v_lo
    out = nc.dram_tensor("out", [128, VC, U1, T], f16, kind="ExternalOutput")

    with tile.TileContext(nc) as tc:
        with tc.tile_pool(name="consts", bufs=1) as cp:
            encT_sb, predT_sb, We_sb, Wp_sb = [], [], [], []
            for c in range(KC):
                t_ = cp.tile([128, T], f16, tag=f"encT{c}")
                nc.sync.dma_start(t_[:], encT[c * 128:(c + 1) * 128, :])
                encT_sb.append(t_)
            for c in range(KC):
                t_ = cp.tile([128, V], f16, tag=f"We{c}")
                nc.sync.dma_start(t_[:], W[c * 128:(c + 1) * 128, :])
                We_sb.append(t_)
            for c in range(KC):
                t_ = cp.tile([128, U1], f16, tag=f"predT{c}")
                nc.sync.dma_start(t_[:], predT[c * 128:(c + 1) * 128, :])
                predT_sb.append(t_)
            for c in range(KC):
                t_ = cp.tile([128, V], f16, tag=f"Wp{c}")
                nc.sync.dma_start(t_[:], W[D + c * 128:D + (c + 1) * 128, :])
                Wp_sb.append(t_)
            bias_sb = cp.tile([1, V], f16, tag="bias")
            nc.sync.dma_start(bias_sb[:], bias[:])
            ones_sb = cp.tile([1, U1], f16, tag="ones")
            nc.sync.dma_start(ones_sb[:], ones[:])

            encP = cp.tile([128, VC * T], f16, tag="encP")      # enc_projT[v, t]
            predP = cp.tile([128, VC * U1], f32, tag="predP")   # pred_projT[v, u] + b[v]

            # ---- projections: V on output partitions ----
            with tc.tile_pool(name="ppool", bufs=2, space="PSUM") as pp:
                for vc in range(VC):
                    vs = slice(vc * 128, (vc + 1) * 128)
                    pse = pp.tile([128, T], f32, tag="pse")
                    for c in range(KC):
                        nc.tensor.matmul(
                            pse[:], We_sb[c][:, vs], encT_sb[c][:],
                            start=(c == 0), stop=(c == KC - 1))
                    nc.scalar.copy(encP[:, vc * T:(vc + 1) * T], pse[:])
                    psp = pp.tile([128, U1], f32, tag="psp")
                    for c in range(KC):
                        nc.tensor.matmul(
                            psp[:], Wp_sb[c][:, vs], predT_sb[c][:],
                            start=(c == 0), stop=False)
                    nc.tensor.matmul(
                        psp[:], bias_sb[0:1, vs], ones_sb[0:1, :],
                        start=False, stop=True)
                    nc.scalar.copy(predP[:, vc * U1:(vc + 1) * U1], psp[:])

            # ---- main loop: per-u scalar-add, big interleaved stores ----
            # Each block's store is split into two vc-halves so the first DMA
            # only waits on half the projections / adds. The per-u adds are
            # split between DVE (tensor_scalar_add, 2x_1p) and the otherwise
            # idle Act engine (activation Identity with per-partition bias).
            NACT = 4  # of the NU u's per (vc, blk), how many go to Act
            with tc.tile_pool(name="outp", bufs=2) as op_:
                for blk in range(NBLK):
                    u0 = blk * NU
                    stage = op_.tile([128, VC, NU, T], f16, tag="stage")
                    for vh in range(2):
                        for vc in range(vh * 4, (vh + 1) * 4):
                            enc_ap = encP[:, vc * T:(vc + 1) * T]
                            for ui in range(NU):
                                u = u0 + ui
                                sc_ap = predP[:, vc * U1 + u:vc * U1 + u + 1]
                                if ui < NU - NACT:
                                    nc.vector.tensor_scalar_add(
                                        stage[:, vc, ui, :], enc_ap, sc_ap)
                                else:
                                    nc.scalar.add(
                                        stage[:, vc, ui, :], enc_ap, sc_ap)
                        nc.sync.dma_start(
                            out[:, vh * 4:(vh + 1) * 4, u0:u0 + NU, :],
                            stage[:, vh * 4:(vh + 1) * 4, :, :])

    nc.compile()
    return nc


def _get_compiled():
    global _COMPILED
    if _COMPILED is None:
        _COMPILED = _build()
    return _COMPILED


def _in_maps(encoder_out, predictor_out, W, b):
    Wc = np.ascontiguousarray(np.asarray(W, dtype=np.float16))
    bias = np.ascontiguousarray(np.asarray(b, dtype=np.float16).reshape(1, V))
    ones = np.ones((1, U1), dtype=np.float16)
    maps = []
    for i in range(B):
        maps.append({
            "encT": np.ascontiguousarray(
                np.asarray(encoder_out[i], dtype=np.float16).T),
            "predT": np.ascontiguousarray(
                np.asarray(predictor_out[i], dtype=np.float16).T),
            "W": Wc,
            "bias": bias,
            "ones": ones,
        })
    return maps


def run(encoder_out, predictor_out, W, b, trace=False, tmpdir=None):
    from concourse.bass_utils import run_bass_kernel_spmd

    nc = _get_compiled()
    maps = _in_maps(encoder_out, predictor_out, W, b)
    res = run_bass_kernel_spmd(
        nc, maps, list(range(B)), trace=trace,
        **({"tmpdir": tmpdir} if tmpdir else {}))
    outs = np.empty((B, T, U1, V), dtype=np.float32)
    for i in range(B):
        arr = res.results[i]["out"]  # [128, VC, U1, T] fp16
        outs[i] = arr.transpose(3, 2, 1, 0).reshape(T, U1, V).astype(np.float32)
    return outs, res


def kernel(encoder_out, predictor_out, W, b):
    outs, _ = run(encoder_out, predictor_out, W, b)
    return outs


# revision 7
# speedup vs baseline: 2.4356x; 1.2027x over previous
"""RNN-T Joiner kernel for Trainium2 (Bass/Tile), 8-core data-parallel over batch.

out[b,t,u,v] = (enc[b,t] @ We)[v] + (pred[b,u] @ Wp)[v] + bias[v]

Layout trick: put V on SBUF partitions. Then for a fixed u, the pred term is a
per-partition scalar, so the broadcast-add is a DVE tensor_scalar_add (2x_1p
fp16 mode) or an Act-engine activation with per-partition bias — no PE one-hot
broadcast and no fp32 tensor_tensor adds. The fp16 datapath halves HBM store
traffic (34 MB/core vs 68 MB), which is the roofline term. Tolerance is 2e-2
rel; fp16 rounding contributes ~6e-4.

Per core (one batch element):
  - Inputs are host-pretiled so each tensor is ONE dma_start with multi-KB
    contiguous runs per partition (128x512B-descriptor loads were taking
    ~20 us before).
  - PE (fp16): enc_projT [v,t] and pred_projT [v,u] with V on output
    partitions; bias folded into pred_projT via a ones-row matmul.
  - Act: PSUM->SBUF evacuation + 4/13 of the per-u adds.
  - DVE: 9/13 of the per-u adds (tensor_scalar_add, fp32 per-partition
    scalar operand is exempt from the 2-byte packing rule).
  - HWDGE DMA: stores with HBM layout [v_lo, u, v_chunk, t] so each store
    half-block has ~25-29 KB contiguous runs; un-permuted on the host.
"""

import sys

sys.path.insert(0, "/opt/trn_rl_repo")

import numpy as np

B, T, U1, D, V = 8, 256, 65, 640, 1024
KC = D // 128   # 5 contraction chunks
VC = V // 128   # 8 vocab chunks
NU = 13         # u's per store block: 5 blocks x 13 = 65
NBLK = U1 // NU
NACT = 4        # of the NU u's per (vc, blk), how many go to Act

_COMPILED = None


def _build():
    import concourse.bacc as bacc
    import concourse.tile as tile
    import concourse.mybir as mybir

    f16 = mybir.dt.float16
    f32 = mybir.dt.float32

    nc = bacc.Bacc("TRN2", target_bir_lowering=False, debug=False, num_devices=8)

    # host-pretiled: encT[p, c, t] = enc.T[c*128+p, t], etc.
    encT = nc.dram_tensor("encT", [128, KC, T], f16, kind="ExternalInput")
    predT = nc.dram_tensor("predT", [128, KC, U1], f16, kind="ExternalInput")
    W = nc.dram_tensor("W", [128, 2 * KC, V], f16, kind="ExternalInput")
    bias = nc.dram_tensor("bias", [1, V], f16, kind="ExternalInput")
    ones = nc.dram_tensor("ones", [1, U1], f16, kind="ExternalInput")
    # out[v_lo, u, v_chunk, t] ; v = v_chunk*128 + v_lo
    out = nc.dram_tensor("out", [128, U1, VC, T], f16, kind="ExternalOutput")

    with tile.TileContext(nc) as tc:
        with tc.tile_pool(name="consts", bufs=1) as cp:
            W_sb = cp.tile([128, 2 * KC, V], f16, tag="W")
            nc.sync.dma_start(W_sb[:], W[:])
            encT_sb = cp.tile([128, KC, T], f16, tag="encT")
            nc.sync.dma_start(encT_sb[:], encT[:])
            predT_sb = cp.tile([128, KC, U1], f16, tag="predT")
            nc.sync.dma_start(predT_sb[:], predT[:])
            bias_sb = cp.tile([1, V], f16, tag="bias")
            nc.sync.dma_start(bias_sb[:], bias[:])
            ones_sb = cp.tile([1, U1], f16, tag="ones")
            nc.sync.dma_start(ones_sb[:], ones[:])

            encP = cp.tile([128, VC * T], f16, tag="encP")      # enc_projT[v, t]
            predP = cp.tile([128, VC * U1], f32, tag="predP")   # pred_projT[v, u] + b[v]

            # ---- projections: V on output partitions ----
            with tc.tile_pool(name="ppool", bufs=2, space="PSUM") as pp:
                for vc in range(VC):
                    vs = slice(vc * 128, (vc + 1) * 128)
                    pse = pp.tile([128, T], f32, tag="pse")
                    for c in range(KC):
                        nc.tensor.matmul(
                            pse[:], W_sb[:, c, vs], encT_sb[:, c, :],
                            start=(c == 0), stop=(c == KC - 1))
                    nc.scalar.copy(encP[:, vc * T:(vc + 1) * T], pse[:])
                    psp = pp.tile([128, U1], f32, tag="psp")
                    for c in range(KC):
                        nc.tensor.matmul(
                            psp[:], W_sb[:, KC + c, vs], predT_sb[:, c, :],
                            start=(c == 0), stop=False)
                    nc.tensor.matmul(
                        psp[:], bias_sb[0:1, vs], ones_sb[0:1, :],
                        start=False, stop=True)
                    nc.scalar.copy(predP[:, vc * U1:(vc + 1) * U1], psp[:])

            # ---- main loop: per-u scalar-add, big interleaved stores ----
            # Each block is stored as two u-halves so the DMA starts after
            # roughly half the block's adds. Per u, the add goes to DVE
            # (tensor_scalar_add) or the Act engine (Identity + bias),
            # interleaved so both engines fill each half concurrently.
            ACT_UI = {NU - NACT + i for i in range(NACT)}  # {9,10,11,12}
            with tc.tile_pool(name="outp", bufs=2) as op_:
                for blk in range(NBLK):
                    u0 = blk * NU
                    stage = op_.tile([128, NU, VC, T], f16, tag="stage")
                    for lo, hi in ((0, 7), (7, NU)):
                        for ui in range(lo, hi):
                            u = u0 + ui
                            for vc in range(VC):
                                enc_ap = encP[:, vc * T:(vc + 1) * T]
                                sc_ap = predP[:, vc * U1 + u:vc * U1 + u + 1]
                                # alternate engines by (ui*VC+vc) parity-ish:
                                if (ui * VC + vc) % NU >= NU - NACT:
                                    nc.scalar.add(
                                        stage[:, ui, vc, :], enc_ap, sc_ap)
                                else:
                                    nc.vector.tensor_scalar_add(
                                        stage[:, ui, vc, :], enc_ap, sc_ap)
                        nc.sync.dma_start(
                            out[:, u0 + lo:u0 + hi, :, :],
                            stage[:, lo:hi, :, :])

    nc.compile()
    return nc


def _get_compiled():
    global _COMPILED
    if _COMPILED is None:
        _COMPILED = _build()
    return _COMPILED


def _in_maps(encoder_out, predictor_out, W, b):
    Wt = np.ascontiguousarray(
        np.asarray(W, dtype=np.float16).reshape(2 * KC, 128, V).transpose(1, 0, 2))
    bias = np.ascontiguousarray(np.asarray(b, dtype=np.float16).reshape(1, V))
    ones = np.ones((1, U1), dtype=np.float16)
    maps = []
    for i in range(B):
        et = np.asarray(encoder_out[i], dtype=np.float16).T  # [D, T]
        pt = np.asarray(predictor_out[i], dtype=np.float16).T  # [D, U1]
        maps.append({
            "encT": np.ascontiguousarray(
                et.reshape(KC, 128, T).transpose(1, 0, 2)),
            "predT": np.ascontiguousarray(
                pt.reshape(KC, 128, U1).transpose(1, 0, 2)),
            "W": Wt,
            "bias": bias,
            "ones": ones,
        })
    return maps


def run(encoder_out, predictor_out, W, b, trace=False, tmpdir=None):
    from concourse.bass_utils import run_bass_kernel_spmd

    nc = _get_compiled()
    maps = _in_maps(encoder_out, predictor_out, W, b)
    res = run_bass_kernel_spmd(
        nc, maps, list(range(B)), trace=trace,
        **({"tmpdir": tmpdir} if tmpdir else {}))
    outs = np.empty((B, T, U1, V), dtype=np.float32)
    for i in range(B):
        arr = res.results[i]["out"]  # [128, U1, VC, T] fp16
        outs[i] = arr.transpose(3, 1, 2, 0).reshape(T, U1, V).astype(np.float32)
    return outs, res


def kernel(encoder_out, predictor_out, W, b):
    outs, _ = run(encoder_out, predictor_out, W, b)
    return outs


# revision 11
# speedup vs baseline: 2.5527x; 1.0481x over previous
"""RNN-T Joiner kernel for Trainium2 (Bass/Tile), 8-core data-parallel over batch.

out[b,t,u,v] = (enc[b,t] @ We)[v] + (pred[b,u] @ Wp)[v] + bias[v]

Layout trick: put V on SBUF partitions. Then for a fixed u, the pred term is a
per-partition scalar, so the broadcast-add is a DVE tensor_scalar_add (2x_1p
fp16 mode) or an Act-engine activation with per-partition bias — no PE one-hot
broadcast and no fp32 tensor_tensor adds. The fp16 datapath halves HBM store
traffic (34 MB/core vs 68 MB), which is the roofline term. Tolerance is 2e-2
rel; fp16 rounding contributes ~6e-4.

Per core (one batch element):
  - Inputs are host-pretiled so each tensor is ONE dma_start with multi-KB
    contiguous runs per partition (128x512B-descriptor loads were taking
    ~20 us before).
  - PE (fp16): enc_projT [v,t] and pred_projT [v,u] with V on output
    partitions; bias folded into pred_projT via a ones-row matmul.
  - Act: PSUM->SBUF evacuation + 4/13 of the per-u adds.
  - DVE: 9/13 of the per-u adds (tensor_scalar_add, fp32 per-partition
    scalar operand is exempt from the 2-byte packing rule).
  - HWDGE DMA: stores with HBM layout [v_lo, u, v_chunk, t] so each store
    half-block has ~25-29 KB contiguous runs; un-permuted on the host.
"""

import sys

sys.path.insert(0, "/opt/trn_rl_repo")

import numpy as np

B, T, U1, D, V = 8, 256, 65, 640, 1024
KC = D // 128   # 5 contraction chunks
VC = V // 128   # 8 vocab chunks
NU = 13         # u's per store block: 5 blocks x 13 = 65
NBLK = U1 // NU
NACT = 4        # of the NU u's per (vc, blk), how many go to Act

_COMPILED = None


def _build():
    import concourse.bacc as bacc
    import concourse.tile as tile
    import concourse.mybir as mybir

    f16 = mybir.dt.float16
    f32 = mybir.dt.float32

    nc = bacc.Bacc("TRN2", target_bir_lowering=False, debug=False, num_devices=8)

    # host-pretiled: encT[p, c, t] = enc.T[c*128+p, t], etc.
    encT = nc.dram_tensor("encT", [128, KC, T], f16, kind="ExternalInput")
    predT = nc.dram_tensor("predT", [128, KC, U1], f16, kind="ExternalInput")
    We = nc.dram_tensor("We", [128, KC, V], f16, kind="ExternalInput")
    Wp = nc.dram_tensor("Wp", [128, KC, V], f16, kind="ExternalInput")
    bias = nc.dram_tensor("bias", [1, V], f16, kind="ExternalInput")
    ones = nc.dram_tensor("ones", [1, U1], f16, kind="ExternalInput")
    # out[v_lo, u, v_chunk, t] ; v = v_chunk*128 + v_lo
    out = nc.dram_tensor("out", [128, U1, VC, T], f16, kind="ExternalOutput")

    with tile.TileContext(nc) as tc:
        with tc.tile_pool(name="consts", bufs=1) as cp:
            # load order = dependency order of the projection matmuls
            encT_sb = cp.tile([128, KC, T], f16, tag="encT")
            nc.sync.dma_start(encT_sb[:], encT[:])
            We_sb = cp.tile([128, KC, V], f16, tag="We")
            nc.sync.dma_start(We_sb[:], We[:])
            predT_sb = cp.tile([128, KC, U1], f16, tag="predT")
            nc.sync.dma_start(predT_sb[:], predT[:])
            Wp_sb = cp.tile([128, KC, V], f16, tag="Wp")
            nc.sync.dma_start(Wp_sb[:], Wp[:])
            bias_sb = cp.tile([1, V], f16, tag="bias")
            nc.sync.dma_start(bias_sb[:], bias[:])
            ones_sb = cp.tile([1, U1], f16, tag="ones")
            nc.sync.dma_start(ones_sb[:], ones[:])

            encP = cp.tile([128, VC * T], f16, tag="encP")      # enc_projT[v, t]
            predP = cp.tile([128, VC * U1], f32, tag="predP")   # pred_projT[v, u] + b[v]

            # ---- projections: V on output partitions ----
            with tc.tile_pool(name="ppool", bufs=2, space="PSUM") as pp:
                for vc in range(VC):
                    vs = slice(vc * 128, (vc + 1) * 128)
                    pse = pp.tile([128, T], f32, tag="pse")
                    for c in range(KC):
                        nc.tensor.matmul(
                            pse[:], We_sb[:, c, vs], encT_sb[:, c, :],
                            start=(c == 0), stop=(c == KC - 1))
                    nc.scalar.copy(encP[:, vc * T:(vc + 1) * T], pse[:])
                    psp = pp.tile([128, U1], f32, tag="psp")
                    for c in range(KC):
                        nc.tensor.matmul(
                            psp[:], Wp_sb[:, c, vs], predT_sb[:, c, :],
                            start=(c == 0), stop=False)
                    nc.tensor.matmul(
                        psp[:], bias_sb[0:1, vs], ones_sb[0:1, :],
                        start=False, stop=True)
                    nc.scalar.copy(predP[:, vc * U1:(vc + 1) * U1], psp[:])

            # ---- main loop: per-u scalar-add, big interleaved stores ----
            # Each block is stored as two u-halves so the DMA starts after
            # roughly half the block's adds. Per u, the add goes to DVE
            # (tensor_scalar_add) or the Act engine (Identity + bias),
            # interleaved so both engines fill each half concurrently.
            with tc.tile_pool(name="outp", bufs=2) as op_:
                for blk in range(NBLK):
                    u0 = blk * NU
                    stage = op_.tile([128, NU, VC, T], f16, tag="stage")
                    if blk == 0:
                        splits = ((0, 3), (3, 7), (7, 10), (10, NU))
                    elif blk == NBLK - 1:
                        splits = ((0, 4), (4, 7), (7, 10), (10, NU))
                    else:
                        splits = ((0, 7), (7, NU))
                    for lo, hi in splits:
                        for ui in range(lo, hi):
                            u = u0 + ui
                            for vc in range(VC):
                                enc_ap = encP[:, vc * T:(vc + 1) * T]
                                sc_ap = predP[:, vc * U1 + u:vc * U1 + u + 1]
                                # alternate engines by (ui*VC+vc) parity-ish:
                                if (ui * VC + vc) % NU >= NU - NACT:
                                    nc.scalar.add(
                                        stage[:, ui, vc, :], enc_ap, sc_ap)
                                else:
                                    nc.vector.tensor_scalar_add(
                                        stage[:, ui, vc, :], enc_ap, sc_ap)
                        nc.sync.dma_start(
                            out[:, u0 + lo:u0 + hi, :, :],
                            stage[:, lo:hi, :, :])

    nc.compile()
    return nc


def _get_compiled():
    global _COMPILED
    if _COMPILED is None:
        _COMPILED = _build()
    return _COMPILED


def _in_maps(encoder_out, predictor_out, W, b):
    Wt = np.asarray(W, dtype=np.float16).reshape(2 * KC, 128, V)
    We = np.ascontiguousarray(Wt[:KC].transpose(1, 0, 2))
    Wp = np.ascontiguousarray(Wt[KC:].transpose(1, 0, 2))
    bias = np.ascontiguousarray(np.asarray(b, dtype=np.float16).reshape(1, V))
    ones = np.ones((1, U1), dtype=np.float16)
    maps = []
    for i in range(B):
        et = np.asarray(encoder_out[i], dtype=np.float16).T  # [D, T]
        pt = np.asarray(predictor_out[i], dtype=np.float16).T  # [D, U1]
        maps.append({
            "encT": np.ascontiguousarray(
                et.reshape(KC, 128, T).transpose(1, 0, 2)),
            "predT": np.ascontiguousarray(
                pt.reshape(KC, 128, U1).transpose(1, 0, 2)),
            "We": We,
            "Wp": Wp,
            "bias": bias,
            "ones": ones,
        })
    return maps


def run(encoder_out, predictor_out, W, b, trace=False, tmpdir=None):
    from concourse.bass_utils import run_bass_kernel_spmd

    nc = _get_compiled()
    maps = _in_maps(encoder_out, predictor_out, W, b)
    res = run_bass_kernel_spmd(
        nc, maps, list(range(B)), trace=trace,
        **({"tmpdir": tmpdir} if tmpdir else {}))
    outs = np.empty((B, T, U1, V), dtype=np.float32)
    for i in range(B):
        arr = res.results[i]["out"]  # [128, U1, VC, T] fp16
        outs[i] = arr.transpose(3, 1, 2, 0).reshape(T, U1, V).astype(np.float32)
    return outs, res


def kernel(encoder_out, predictor_out, W, b):
    outs, _ = run(encoder_out, predictor_out, W, b)
    return outs
